# revision 2
# baseline (speedup 1.0000x reference)
"""EqualizedOddsLoss on 8 TRN2 NeuronCores — v2 (CDF-ladder scheme).

Pack each element into t = 4*gid + 2*binp + lab in [0, 32) (fp16), then count
elements above each of 31 half-integer thresholds with single-source
tensor_scalar(is_gt)+accumulate passes (DVE, 4x/2x perf mode) and
activation(Sign)+accumulate passes (ACT engine) running concurrently.
Host differences the CDF ladder into the 32-bin joint histogram per core,
then finishes the tiny TPR/FPR pairwise reduction.

binp = (sigmoid(pred) > 0.5) = (pred > 0) exactly (reference applies sigmoid
because randn predictions fall outside [0,1]).
"""

import numpy as np

import concourse.bass as bass
import concourse.bacc as bacc
import concourse.mybir as mybir
import concourse.tile as tile
from concourse.bass_utils import run_bass_kernel_spmd

B = 16777216
G = 8
EPS = 1e-08
WEIGHT = 1.0
N_CORES = 8
NPC = B // N_CORES                 # 2,097,152 per core
P = 128
F = 4096                           # free-dim per tile
T = NPC // (P * F)                 # 4 tiles
NCUT = 31                          # thresholds 0.5 .. 30.5
N_ACT = 17                         # cuts on ACT (Sign); rest on DVE
N_DVE = NCUT - N_ACT
# cut k (threshold k+0.5): k < N_DVE -> DVE, else ACT
USE_CAST_GID = True                # SWDGE dma cast i32 -> f16
USE_CAST_LAB = True                # SWDGE dma cast f32 -> f16

_CACHE = {}


def _build():
    nc = bacc.Bacc("TRN2", target_bir_lowering=False, debug=False)
    f32 = mybir.dt.float32
    f16 = mybir.dt.float16
    i32 = mybir.dt.int32
    Alu = mybir.AluOpType
    Act = mybir.ActivationFunctionType

    pred_ext = nc.declare_dram_parameter("predictions", [NPC, 1], f32, isOutput=False)
    lab_ext = nc.declare_dram_parameter("labels", [NPC, 1], f32, isOutput=False)
    gid_ext = nc.declare_dram_parameter("protected_attributes", [NPC, 1], i32, isOutput=False)
    out_dve = nc.declare_dram_parameter("out_dve", [P, T * N_DVE], f32, isOutput=True)
    out_act = nc.declare_dram_parameter("out_act", [P, T * N_ACT], f32, isOutput=True)

    pred_v = pred_ext[:, :].rearrange("(t p f) o -> t p (f o)", t=T, p=P, f=F)
    lab_v = lab_ext[:, :].rearrange("(t p f) o -> t p (f o)", t=T, p=P, f=F)
    gid_v = gid_ext[:, :].rearrange("(t p f) o -> t p (f o)", t=T, p=P, f=F)

    with tile.TileContext(nc) as tc:
        with (
            tc.tile_pool(name="io", bufs=2) as io_pool,
            tc.tile_pool(name="work", bufs=2) as work_pool,
            tc.tile_pool(name="scr", bufs=1) as scr_pool,
            tc.tile_pool(name="ps", bufs=1, space="PSUM") as ps_pool,
        ):
            acc_d = scr_pool.tile([P, T * N_DVE], f32, tag="acc_d")
            acc_a = scr_pool.tile([P, T * N_ACT], f32, tag="acc_a")
            o_dve = scr_pool.tile([P, F], f16, tag="o_dve")
            o_act = ps_pool.tile([P, F], f32, tag="o_act")
            bias = scr_pool.tile([P, N_ACT], f32, tag="bias")
            for j in range(N_ACT):
                kcut = N_DVE + j
                nc.vector.memset(bias[:, j: j + 1], -(kcut + 0.5))

            for t in range(T):
                pred = io_pool.tile([P, F], f16, tag="pred")
                lab16 = io_pool.tile([P, F], f16, tag="lab")
                gid16 = io_pool.tile([P, F], f16, tag="gid")
                nc.gpsimd.dma_start(pred[:], pred_v[t, :, :])
                nc.gpsimd.dma_start(lab16[:], lab_v[t, :, :])
                nc.gpsimd.dma_start(gid16[:], gid_v[t, :, :])

                binp2 = work_pool.tile([P, F], f16, tag="binp2")
                v = work_pool.tile([P, F], f16, tag="v")
                gid4 = work_pool.tile([P, F], f16, tag="gid4")
                tt = work_pool.tile([P, F], f16, tag="tt")

                # binp2 = 2*(pred > 0)   [f16 single-src, 4x]
                nc.vector.tensor_scalar(
                    binp2[:], pred[:], 0.0, 2.0, op0=Alu.is_gt, op1=Alu.mult
                )
                # v = lab + 2*binp      [f16 TT, 2x_1P]
                nc.vector.tensor_tensor(v[:], lab16[:], binp2[:], op=Alu.add)
                # gid4 = 4*gid          [f16 single-src, 4x]
                nc.vector.tensor_scalar(gid4[:], gid16[:], 4.0, None, op0=Alu.mult)
                # tt = gid4 + v         [f16 TT, 2x_1P]
                nc.vector.tensor_tensor(tt[:], gid4[:], v[:], op=Alu.add)

                for k in range(N_DVE):
                    nc.vector.tensor_scalar(
                        o_dve[:], tt[:], k + 0.5, 0.0,
                        op0=Alu.is_gt, op1=Alu.add,
                        accum_out=acc_d[:, t * N_DVE + k: t * N_DVE + k + 1],
                    )
                for j in range(N_ACT):
                    nc.scalar.activation(
                        o_act[:], tt[:], Act.Sign,
                        bias=bias[:, j: j + 1], scale=1.0,
                        accum_out=acc_a[:, t * N_ACT + j: t * N_ACT + j + 1],
                    )

            nc.sync.dma_start(out_dve[:, :], acc_d[:])
            nc.sync.dma_start(out_act[:, :], acc_a[:])
    nc.compile()
    return nc


def _get_nc():
    if "nc" not in _CACHE:
        _CACHE["nc"] = _build()
    return _CACHE["nc"]


def kernel(predictions, labels, protected_attributes, num_groups):
    num_groups = int(num_groups)
    assert num_groups == G and predictions.shape[0] == B

    pred = np.ascontiguousarray(predictions, dtype=np.float32)
    lab = np.ascontiguousarray(labels, dtype=np.float32)
    gid = np.ascontiguousarray(protected_attributes, dtype=np.int32)

    in_maps = []
    for c in range(N_CORES):
        s = slice(c * NPC, (c + 1) * NPC)
        in_maps.append(
            {
                "predictions": pred[s],
                "labels": lab[s],
                "protected_attributes": gid[s],
            }
        )

    nc = _get_nc()
    res = run_bass_kernel_spmd(nc, in_maps, core_ids=list(range(N_CORES)))
    outs = res.results if hasattr(res, "results") else res

    # Host finish: assemble the CDF ladder. C[k] = #{t > k+0.5}, k=0..30.
    C = np.zeros(NCUT, dtype=np.float64)
    for c in range(N_CORES):
        a_d = np.asarray(outs[c]["out_dve"], dtype=np.float64)  # [P, T*N_DVE]
        a_a = np.asarray(outs[c]["out_act"], dtype=np.float64)  # [P, T*N_ACT]
        a_d = a_d.sum(axis=0).reshape(T, N_DVE).sum(axis=0)
        a_a = a_a.sum(axis=0).reshape(T, N_ACT).sum(axis=0)
        C[:N_DVE] += a_d
        # Sign pass: sum = 2*C_gt - count  ->  C_gt = (sum + count)/2
        n_per_col = float(P * F) * T
        C[N_DVE:] += (a_a + n_per_col) / 2.0

    n = np.zeros(32, dtype=np.float64)      # joint counts, level = 4g+2b+l
    n[0] = B - C[0]
    for k in range(1, 31):
        n[k] = C[k - 1] - C[k]
    n[31] = C[30]

    lv = n.reshape(G, 4)                    # [g, 2*binp+lab]
    s_lab = lv[:, 1] + lv[:, 3]
    s_binp = lv[:, 2] + lv[:, 3]
    s_tp = lv[:, 3]

    tp = s_tp
    pos = s_lab
    fp = s_binp - s_tp
    neg = B - pos
    tpr = tp / (pos + EPS)
    fpr = fp / (neg + EPS)
    d = np.abs(tpr[:, None] - tpr[None, :]) + np.abs(fpr[:, None] - fpr[None, :])
    iu = np.triu(np.ones((G, G), dtype=bool), k=1)
    total = np.sum(np.where(iu, d, 0.0))
    return np.float32(WEIGHT * total)


# revision 3
# speedup vs baseline: 1.1427x; 1.1427x over previous
"""EqualizedOddsLoss on 8 TRN2 NeuronCores — v2 (CDF-ladder scheme).

Pack each element into t = 4*gid + 2*binp + lab in [0, 32) (fp16), then count
elements above each of 31 half-integer thresholds with single-source
tensor_scalar(is_gt)+accumulate passes (DVE, 4x/2x perf mode) and
activation(Sign)+accumulate passes (ACT engine) running concurrently.
Host differences the CDF ladder into the 32-bin joint histogram per core,
then finishes the tiny TPR/FPR pairwise reduction.

binp = (sigmoid(pred) > 0.5) = (pred > 0) exactly (reference applies sigmoid
because randn predictions fall outside [0,1]).
"""

import numpy as np

import concourse.bass as bass
import concourse.bacc as bacc
import concourse.mybir as mybir
import concourse.tile as tile
from concourse.bass_utils import run_bass_kernel_spmd

B = 16777216
G = 8
EPS = 1e-08
WEIGHT = 1.0
N_CORES = 8
NPC = B // N_CORES                 # 2,097,152 per core
P = 128
F = 4096                           # free-dim per tile
T = NPC // (P * F)                 # 4 tiles
NCUT = 31                          # thresholds 0.5 .. 30.5
N_ACT = 17                         # cuts on ACT (Sign); rest on DVE
N_DVE = NCUT - N_ACT
# cut k (threshold k+0.5): k < N_DVE -> DVE, else ACT
USE_CAST_GID = True                # SWDGE dma cast i32 -> f16
USE_CAST_LAB = True                # SWDGE dma cast f32 -> f16

_CACHE = {}


def _build():
    nc = bacc.Bacc("TRN2", target_bir_lowering=False, debug=False)
    f32 = mybir.dt.float32
    f16 = mybir.dt.float16
    i32 = mybir.dt.int32
    Alu = mybir.AluOpType
    Act = mybir.ActivationFunctionType

    pred_ext = nc.declare_dram_parameter("predictions", [NPC, 1], f32, isOutput=False)
    lab_ext = nc.declare_dram_parameter("labels", [NPC, 1], f32, isOutput=False)
    gid_ext = nc.declare_dram_parameter("protected_attributes", [NPC, 1], i32, isOutput=False)
    out_dve = nc.declare_dram_parameter("out_dve", [P, T * N_DVE], f32, isOutput=True)
    out_act = nc.declare_dram_parameter("out_act", [P, T * N_ACT], f32, isOutput=True)

    pred_v = pred_ext[:, :].rearrange("(t p f) o -> t p (f o)", t=T, p=P, f=F)
    lab_v = lab_ext[:, :].rearrange("(t p f) o -> t p (f o)", t=T, p=P, f=F)
    gid_v = gid_ext[:, :].rearrange("(t p f) o -> t p (f o)", t=T, p=P, f=F)

    with tile.TileContext(nc) as tc:
        with (
            tc.tile_pool(name="io", bufs=2) as io_pool,
            tc.tile_pool(name="work", bufs=2) as work_pool,
            tc.tile_pool(name="scr", bufs=1) as scr_pool,
            tc.tile_pool(name="ps", bufs=1, space="PSUM") as ps_pool,
        ):
            acc_d = scr_pool.tile([P, T * N_DVE], f32, tag="acc_d")
            acc_a = scr_pool.tile([P, T * N_ACT], f32, tag="acc_a")
            o_dve = scr_pool.tile([P, 1], f16, tag="o_dve")
            o_act = ps_pool.tile([P, F], f32, tag="o_act")
            bias = scr_pool.tile([P, N_ACT], f32, tag="bias")
            for j in range(N_ACT):
                kcut = N_DVE + j
                nc.vector.memset(bias[:, j: j + 1], -(kcut + 0.5))

            for t in range(T):
                pred = io_pool.tile([P, F], f16, tag="pred")
                lab16 = io_pool.tile([P, F], f16, tag="lab")
                gid16 = io_pool.tile([P, F], f16, tag="gid")
                nc.gpsimd.dma_start(pred[:], pred_v[t, :, :])
                nc.gpsimd.dma_start(lab16[:], lab_v[t, :, :])
                nc.gpsimd.dma_start(gid16[:], gid_v[t, :, :])

                binp2 = work_pool.tile([P, F], f16, tag="binp2")
                v = work_pool.tile([P, F], f16, tag="v")
                gid4 = work_pool.tile([P, F], f16, tag="gid4")
                tt = work_pool.tile([P, F], f16, tag="tt")

                # binp2 = 2*(pred > 0)   [f16 single-src, 4x]
                nc.vector.tensor_scalar(
                    binp2[:], pred[:], 0.0, 2.0, op0=Alu.is_gt, op1=Alu.mult
                )
                # v = lab + 2*binp      [f16 TT, 2x_1P]
                nc.vector.tensor_tensor(v[:], lab16[:], binp2[:], op=Alu.add)
                # gid4 = 4*gid          [f16 single-src, 4x]
                nc.vector.tensor_scalar(gid4[:], gid16[:], 4.0, None, op0=Alu.mult)
                # tt = gid4 + v         [f16 TT, 2x_1P]
                nc.vector.tensor_tensor(tt[:], gid4[:], v[:], op=Alu.add)

                for k in range(N_DVE):
                    nc.vector.tensor_scalar(
                        o_dve[:].to_broadcast((P, F)), tt[:], k + 0.5, 0.0,
                        op0=Alu.is_gt, op1=Alu.add,
                        accum_out=acc_d[:, t * N_DVE + k: t * N_DVE + k + 1],
                    )
                for j in range(N_ACT):
                    nc.scalar.activation(
                        o_act[:], tt[:], Act.Sign,
                        bias=bias[:, j: j + 1], scale=1.0,
                        accum_out=acc_a[:, t * N_ACT + j: t * N_ACT + j + 1],
                    )

            nc.sync.dma_start(out_dve[:, :], acc_d[:])
            nc.sync.dma_start(out_act[:, :], acc_a[:])
    nc.compile()
    return nc


def _get_nc():
    if "nc" not in _CACHE:
        _CACHE["nc"] = _build()
    return _CACHE["nc"]


def kernel(predictions, labels, protected_attributes, num_groups):
    num_groups = int(num_groups)
    assert num_groups == G and predictions.shape[0] == B

    pred = np.ascontiguousarray(predictions, dtype=np.float32)
    lab = np.ascontiguousarray(labels, dtype=np.float32)
    gid = np.ascontiguousarray(protected_attributes, dtype=np.int32)

    in_maps = []
    for c in range(N_CORES):
        s = slice(c * NPC, (c + 1) * NPC)
        in_maps.append(
            {
                "predictions": pred[s],
                "labels": lab[s],
                "protected_attributes": gid[s],
            }
        )

    nc = _get_nc()
    res = run_bass_kernel_spmd(nc, in_maps, core_ids=list(range(N_CORES)))
    outs = res.results if hasattr(res, "results") else res

    # Host finish: assemble the CDF ladder. C[k] = #{t > k+0.5}, k=0..30.
    C = np.zeros(NCUT, dtype=np.float64)
    for c in range(N_CORES):
        a_d = np.asarray(outs[c]["out_dve"], dtype=np.float64)  # [P, T*N_DVE]
        a_a = np.asarray(outs[c]["out_act"], dtype=np.float64)  # [P, T*N_ACT]
        a_d = a_d.sum(axis=0).reshape(T, N_DVE).sum(axis=0)
        a_a = a_a.sum(axis=0).reshape(T, N_ACT).sum(axis=0)
        C[:N_DVE] += a_d
        # Sign pass: sum = 2*C_gt - count  ->  C_gt = (sum + count)/2
        n_per_col = float(P * F) * T
        C[N_DVE:] += (a_a + n_per_col) / 2.0

    n = np.zeros(32, dtype=np.float64)      # joint counts, level = 4g+2b+l
    n[0] = B - C[0]
    for k in range(1, 31):
        n[k] = C[k - 1] - C[k]
    n[31] = C[30]

    lv = n.reshape(G, 4)                    # [g, 2*binp+lab]
    s_lab = lv[:, 1] + lv[:, 3]
    s_binp = lv[:, 2] + lv[:, 3]
    s_tp = lv[:, 3]

    tp = s_tp
    pos = s_lab
    fp = s_binp - s_tp
    neg = B - pos
    tpr = tp / (pos + EPS)
    fpr = fp / (neg + EPS)
    d = np.abs(tpr[:, None] - tpr[None, :]) + np.abs(fpr[:, None] - fpr[None, :])
    iu = np.triu(np.ones((G, G), dtype=bool), k=1)
    total = np.sum(np.where(iu, d, 0.0))
    return np.float32(WEIGHT * total)


# revision 4
# speedup vs baseline: 1.1431x; 1.0003x over previous
"""EqualizedOddsLoss on 8 TRN2 NeuronCores — v6 (moment-knot hybrid).

t = 4*gid + 2*binp + lab in [0,32) packed fp16. Histogram recovered from:
- ACT engine: 15 Sign+accumulate cuts at thresholds 16.5..30.5 -> bins 17..31.
- DVE: 8 "knots" c in {0,2,..,14}: u_c = relu(t-c) (tensor_scalar, 4x mode)
  then bn_stats over 512-element chunks, giving Sum(u_c) and Sum(u_c^2) —
  TWO independent histogram functionals per knot-stream. Solving the
  block-triangular system top-down recovers bins 1..16 (two bins per knot).
Host does the tiny solve + TPR/FPR pairwise reduction in float64.

binp = (sigmoid(pred) > 0.5) = (pred > 0) exactly for this input family.
"""

import numpy as np

import concourse.bass as bass
import concourse.bacc as bacc
import concourse.mybir as mybir
import concourse.tile as tile
from concourse.bass_utils import run_bass_kernel_spmd

B = 16777216
G = 8
EPS = 1e-08
WEIGHT = 1.0
N_CORES = 8
NPC = B // N_CORES                 # 2,097,152 per core
P = 128
F = 4096                           # free-dim per tile
T = NPC // (P * F)                 # 4 tiles
CH = 512                           # bn_stats hardware max free size
NCH = F // CH                      # 8 chunks per tile
KNOTS = [0, 2, 4, 6, 8, 10, 12, 14]
K = len(KNOTS)
N_ACT = 15                         # Sign cuts at 16.5 .. 30.5

_CACHE = {}


def _build():
    nc = bacc.Bacc("TRN2", target_bir_lowering=False, debug=False)
    f32 = mybir.dt.float32
    f16 = mybir.dt.float16
    i32 = mybir.dt.int32
    Alu = mybir.AluOpType
    Act = mybir.ActivationFunctionType

    pred_ext = nc.declare_dram_parameter("predictions", [NPC, 1], f32, isOutput=False)
    lab_ext = nc.declare_dram_parameter("labels", [NPC, 1], f32, isOutput=False)
    gid_ext = nc.declare_dram_parameter("protected_attributes", [NPC, 1], i32, isOutput=False)
    out_bn = nc.declare_dram_parameter("out_bn", [P, T * K * NCH * 6], f32, isOutput=True)
    out_act = nc.declare_dram_parameter("out_act", [P, T * N_ACT], f32, isOutput=True)

    pred_v = pred_ext[:, :].rearrange("(t p f) o -> t p (f o)", t=T, p=P, f=F)
    lab_v = lab_ext[:, :].rearrange("(t p f) o -> t p (f o)", t=T, p=P, f=F)
    gid_v = gid_ext[:, :].rearrange("(t p f) o -> t p (f o)", t=T, p=P, f=F)

    with tile.TileContext(nc) as tc:
        with (
            tc.tile_pool(name="io", bufs=2) as io_pool,
            tc.tile_pool(name="work", bufs=2) as work_pool,
            tc.tile_pool(name="scr", bufs=1) as scr_pool,
            tc.tile_pool(name="ps", bufs=1, space="PSUM") as ps_pool,
        ):
            st = scr_pool.tile([P, T * K * NCH * 6], f32, tag="st")
            acc_a = scr_pool.tile([P, T * N_ACT], f32, tag="acc_a")
            o_act = ps_pool.tile([P, F], f32, tag="o_act")
            bias = scr_pool.tile([P, N_ACT], f32, tag="bias")
            for j in range(N_ACT):
                nc.vector.memset(bias[:, j: j + 1], -(16 + j + 0.5))

            for t in range(T):
                pred = io_pool.tile([P, F], f16, tag="pred")
                lab16 = io_pool.tile([P, F], f16, tag="lab")
                gid16 = io_pool.tile([P, F], f16, tag="gid")
                nc.gpsimd.dma_start(pred[:], pred_v[t, :, :])
                nc.gpsimd.dma_start(lab16[:], lab_v[t, :, :])
                nc.gpsimd.dma_start(gid16[:], gid_v[t, :, :])

                binp2 = work_pool.tile([P, F], f16, tag="binp2")
                v = work_pool.tile([P, F], f16, tag="v")
                gid4 = work_pool.tile([P, F], f16, tag="gid4")
                tt = work_pool.tile([P, F], f16, tag="tt")
                u = work_pool.tile([P, F], f16, tag="u")

                nc.vector.tensor_scalar(
                    binp2[:], pred[:], 0.0, 2.0, op0=Alu.is_gt, op1=Alu.mult
                )
                nc.vector.tensor_tensor(v[:], lab16[:], binp2[:], op=Alu.add)
                nc.vector.tensor_scalar(gid4[:], gid16[:], 4.0, None, op0=Alu.mult)
                nc.vector.tensor_tensor(tt[:], gid4[:], v[:], op=Alu.add)

                # ACT sign cuts (bins 17..31)
                for j in range(N_ACT):
                    nc.scalar.activation(
                        o_act[:], tt[:], Act.Sign,
                        bias=bias[:, j: j + 1], scale=1.0,
                        accum_out=acc_a[:, t * N_ACT + j: t * N_ACT + j + 1],
                    )

                # DVE moment knots (bins 1..16)
                for kn, c in enumerate(KNOTS):
                    nc.vector.tensor_scalar(
                        u[:], tt[:], float(c), 0.0,
                        op0=Alu.subtract, op1=Alu.max,
                    )
                    for ch in range(NCH):
                        off = ((t * K + kn) * NCH + ch) * 6
                        nc.vector.bn_stats(
                            st[:, off: off + 6],
                            u[:, ch * CH: (ch + 1) * CH],
                        )

            nc.sync.dma_start(out_bn[:, :], st[:])
            nc.sync.dma_start(out_act[:, :], acc_a[:])
    nc.compile()
    return nc


def _get_nc():
    if "nc" not in _CACHE:
        _CACHE["nc"] = _build()
    return _CACHE["nc"]


def kernel(predictions, labels, protected_attributes, num_groups):
    num_groups = int(num_groups)
    assert num_groups == G and predictions.shape[0] == B

    pred = np.ascontiguousarray(predictions, dtype=np.float32)
    lab = np.ascontiguousarray(labels, dtype=np.float32)
    gid = np.ascontiguousarray(protected_attributes, dtype=np.int32)

    in_maps = []
    for c in range(N_CORES):
        s = slice(c * NPC, (c + 1) * NPC)
        in_maps.append(
            {
                "predictions": pred[s],
                "labels": lab[s],
                "protected_attributes": gid[s],
            }
        )

    nc = _get_nc()
    res = run_bass_kernel_spmd(nc, in_maps, core_ids=list(range(N_CORES)))
    outs = res.results if hasattr(res, "results") else res

    # ---- host finish ----
    # ACT: C[k] = #{t > k+0.5} for k = 16..30
    C_act = np.zeros(N_ACT, dtype=np.float64)
    R = np.zeros(K, dtype=np.float64)   # R[kn] = sum relu(t - c_kn)
    Q = np.zeros(K, dtype=np.float64)   # Q[kn] = sum relu(t - c_kn)^2
    n_tile = float(P * F)
    half = float(CH // 2)
    for c in range(N_CORES):
        a_a = np.asarray(outs[c]["out_act"], dtype=np.float64).sum(axis=0)
        aa = a_a.reshape(T, N_ACT).sum(axis=0)
        C_act += (aa + n_tile * T) / 2.0
        bn = np.asarray(outs[c]["out_bn"], dtype=np.float64).reshape(P, T, K, NCH, 6)
        me, m2e = bn[..., 1], bn[..., 2]
        mo, m2o = bn[..., 4], bn[..., 5]
        R += (half * (me + mo)).sum(axis=(0, 1, 3))
        Q += (m2e + half * me * me + m2o + half * mo * mo).sum(axis=(0, 1, 3))

    n = np.zeros(32, dtype=np.float64)
    # bins 17..31 from ACT ladder: C_act[j] = C(16.5 + j)
    n[31] = C_act[14]
    for k in range(17, 31):
        n[k] = C_act[k - 17] - C_act[k - 16]
    # bins 1..16 from knots, top-down (two per knot)
    for kn in range(K - 1, -1, -1):
        ck = KNOTS[kn]
        ks = np.arange(ck + 3, 32)
        Rres = R[kn] - np.sum((ks - ck) * n[ks])
        Qres = Q[kn] - np.sum((ks - ck) ** 2 * n[ks])
        n[ck + 2] = (Qres - Rres) / 2.0
        n[ck + 1] = Rres - 2.0 * n[ck + 2]
    n[0] = B - n[1:].sum()

    lv = n.reshape(G, 4)                    # [g, 2*binp+lab]
    s_lab = lv[:, 1] + lv[:, 3]
    s_binp = lv[:, 2] + lv[:, 3]
    s_tp = lv[:, 3]

    tp = s_tp
    pos = s_lab
    fp = s_binp - s_tp
    neg = B - pos
    tpr = tp / (pos + EPS)
    fpr = fp / (neg + EPS)
    d = np.abs(tpr[:, None] - tpr[None, :]) + np.abs(fpr[:, None] - fpr[None, :])
    iu = np.triu(np.ones((G, G), dtype=bool), k=1)
    total = np.sum(np.where(iu, d, 0.0))
    return np.float32(WEIGHT * total)
